# revision 29
# baseline (speedup 1.0000x reference)
"""DGCNN forward on 8 Trainium2 NeuronCores, data-parallel over batch.

Per core (one point cloud, x [3, 2048]):
  4 edge-conv blocks, each:
    s[n,m] = 2*x_n.x_m - |x_m|^2            (augmented fp32 PE matmul; row-constant
                                             -|x_n|^2 dropped: rank-invariant per row)
    top-20 of each s row:  3 rounds of (max8, max_index, match_replace)
    x' = lrelu(max_k A[:, idx_k] + B)       (A = Wn@x, B = (Wc-Wn)@x; edge features
                                             never materialized: conv is linear and
                                             lrelu/max commute)
    The k=0 neighbor is always the point itself (s[n,n] is the row max), so its
    A-row comes from one contiguous DMA; only k=1..19 use indirect gathers.
    Block 4's A-table is bf16 (its output feeds only the global conv, no further
    knn), halving its gather traffic and enabling the 2x DVE mode for its k-max.
  then g = lrelu(Wg @ concat(x1..x4)), out = max_n g.

knn grams and blocks 1-3 stay exact fp32 (noise there corrupts the knn sets).
"""

import numpy as np
from contextlib import ExitStack

import concourse.bass as bass
import concourse.bacc as bacc
import concourse.mybir as mybir
from concourse.bass import IndirectOffsetOnAxis
from concourse.tile import TileContext

F32 = mybir.dt.float32
BF16 = mybir.dt.bfloat16
U16 = mybir.dt.uint16
U32 = mybir.dt.uint32

B, N, KNN, P = 8, 2048, 20, 128
NCHUNK = N // P            # 16
NEG = -3.0e38
SLOPE = 0.2
BLOCKS = [(3, 64), (64, 64), (64, 128), (128, 256)]
ACT = mybir.ActivationFunctionType
ALU = mybir.AluOpType


def build_nc():
    nc = bacc.Bacc("TRN2", target_bir_lowering=False)

    x_in = nc.dram_tensor("x", [3, N], F32, kind="ExternalInput")
    w_in = {}
    for bi, (c, o) in enumerate(BLOCKS):
        w_in[f"wnt{bi}"] = nc.dram_tensor(f"wnt{bi}", [c, o], F32, kind="ExternalInput")
        w_in[f"wdt{bi}"] = nc.dram_tensor(f"wdt{bi}", [c, o], F32, kind="ExternalInput")
    w_in["wgt"] = nc.dram_tensor("wgt", [512, 1024], F32, kind="ExternalInput")
    id_in = nc.dram_tensor("ident", [P, P], F32, kind="ExternalInput")
    out_d = nc.dram_tensor("out", [1024, 1], F32, kind="ExternalOutput")

    # DRAM scratch: per-block A^T feature tables
    at_dram = [
        nc.dram_tensor(f"at{bi}_scratch", [N, o], BF16 if bi == 3 else F32,
                       kind="Internal")
        for bi, (_, o) in enumerate(BLOCKS)
    ]

    with TileContext(nc) as tc, ExitStack() as ctx:
        ep = ctx.enter_context            # shorthand
        const = ep(tc.tile_pool(name="const", bufs=1))
        wpool = ep(tc.tile_pool(name="weights", bufs=1))
        xpool = ep(tc.tile_pool(name="xtiles", bufs=1))
        bpool = ep(tc.tile_pool(name="blockp", bufs=1))
        spool = ep(tc.tile_pool(name="sbuf", bufs=2))
        tkpool = ep(tc.tile_pool(name="topk", bufs=2))
        gpool = ep(tc.tile_pool(name="gather", bufs=2))
        perb = ep(tc.tile_pool(name="perblock", bufs=1))
        pp_s = ep(tc.tile_pool(name="ps_s", bufs=2, space="PSUM"))
        pp_m = ep(tc.tile_pool(name="ps_m", bufs=2, space="PSUM"))
        pp_t = ep(tc.tile_pool(name="ps_t", bufs=2, space="PSUM"))

        # block input x0 — first DMA: everything in block 1 hangs off it
        x0 = xpool.tile([3, N], F32)
        nc.sync.dma_start(out=x0[:], in_=x_in[:])

        # ---- constants ----
        ident = const.tile([P, P], F32)
        nc.sync.dma_start(out=ident[:], in_=id_in[:])
        ones_col = const.tile([P, 1], F32)
        nc.vector.memset(ones_col[:], 1.0)
        ones_row = const.tile([1, P], F32)
        nc.vector.memset(ones_row[:], 1.0)

        # ---- load pre-transposed weights (host supplies WnT/WdT/WgT) ----
        wnT, wdT = [], []
        for bi, (c, o) in enumerate(BLOCKS):
            wn = wpool.tile([c, o], F32, tag=f"wn{bi}")
            nc.sync.dma_start(out=wn[:], in_=w_in[f"wnt{bi}"][:])
            wd = wpool.tile([c, o], F32, tag=f"wd{bi}")
            nc.sync.dma_start(out=wd[:], in_=w_in[f"wdt{bi}"][:])
            wnT.append(wn)
            wdT.append(wd)

        # WgT loads are deferred into block 2 (first needed by block 4's gconv)
        wgT = [wpool.tile([P, 1024], F32, tag=f"wg{k}", name=f"wgT{k}") for k in range(4)]

        # ---- x_cat tiles (c12 assembled from x1t/x2t before the global conv) ----
        c12 = xpool.tile([P, N], F32)
        c3 = xpool.tile([P, N], F32)
        c4a = xpool.tile([P, N], F32)
        c4b = xpool.tile([P, N], F32)
        x1t = xpool.tile([64, N], F32)
        x2t = xpool.tile([64, N], F32)

        # global-conv partial maxima, filled as block 4 quarters complete
        redv_all = perb.tile([P, 32], F32, name="redv_all")

        def emit_gconv_unit(nq, oc):
            xcat = [c12, c3, c4a, c4b]
            pg = pp_s.tile([P, 512], F32, space="PSUM", tag="ph")
            for k in range(4):
                nc.tensor.matmul(out=pg[:], lhsT=wgT[k][:, oc * P:(oc + 1) * P],
                                 rhs=xcat[k][:, nq * 512:(nq + 1) * 512],
                                 start=(k == 0), stop=(k == 3))
            nc.vector.tensor_reduce(out=redv_all[:, oc * 4 + nq:oc * 4 + nq + 1],
                                    in_=pg[:], axis=mybir.AxisListType.X,
                                    op=ALU.max)

        def block_input(bi):
            return [x0[:], x1t[:], x2t[:], c3[:]][bi]

        def block_out_dst(bi):
            return [[x1t], [x2t], [c3], [c4a, c4b]][bi]

        gconv_pending = []

        # ================= edge blocks =================
        for bi, (C, O) in enumerate(BLOCKS):
            xb = block_input(bi)
            at_dt = BF16 if bi == 3 else F32

            if bi == 1:
                for k in range(4):
                    nc.sync.dma_start(out=wgT[k][:],
                                      in_=w_in["wgt"][k * P:(k + 1) * P, :])

            # --- per-block prep: xsq first (it feeds the negxx matmuls) ---
            xsq = bpool.tile([C, N], F32, tag="xsq")
            nc.scalar.activation(out=xsq[:], in_=xb, func=ACT.Square)
            x2 = bpool.tile([C, N], F32, tag="x2")
            nc.scalar.activation(out=x2[:], in_=xb, func=ACT.Copy, scale=2.0)
            negxx = bpool.tile([1, N], F32, tag="negxx")
            for q in range(4):
                mq = pp_m.tile([1, 512], F32, space="PSUM", tag="mm")
                nc.tensor.matmul(out=mq[:], lhsT=ones_col[:C, :], rhs=xsq[:, q * 512:(q + 1) * 512],
                                 start=True, stop=True)
                nc.scalar.activation(out=negxx[:, q * 512:(q + 1) * 512], in_=mq[:],
                                     func=ACT.Copy, scale=-1.0)

            def emit_at_table():
                # A^T rows to DRAM: at[n, :] = x_n . WnT  (chunk-wise).
                # Emitted after the first grams: it only gates the gathers.
                for i in range(NCHUNK):
                    ap_ = pp_m.tile([P, O], F32, space="PSUM", tag="mm")
                    nc.tensor.matmul(out=ap_[:], lhsT=xb[:, i * P:(i + 1) * P],
                                     rhs=wnT[bi][:], start=True, stop=True)
                    at_sb = spool.tile([P, O], at_dt, tag="at_sb")
                    nc.scalar.copy(out=at_sb[:], in_=ap_[:])
                    nc.sync.dma_start(out=at_dram[bi][i * P:(i + 1) * P, :], in_=at_sb[:])

            # augmented gram lhs/rhs for C <= 64 (single fused matmul); block4 separate
            if C <= 64:
                # augmented row must start at a 32-aligned partition; pad with zeros
                cpad = ((C + 31) // 32) * 32
                gl = bpool.tile([cpad + 1, N], F32, tag="gramL")
                gr = bpool.tile([cpad + 1, N], F32, tag="gramR")
                if cpad != C:
                    nc.vector.memset(gl[:], 0.0)
                    nc.vector.memset(gr[:], 0.0)
                nc.scalar.copy(out=gl[:C, :], in_=xb)
                nc.vector.memset(gl[cpad:cpad + 1, :], 1.0)
                nc.vector.tensor_copy(out=gr[:C, :], in_=x2[:])
                # engines are lane-aligned: partition 0 -> cpad needs a DMA
                nc.sync.dma_start(out=gr[cpad:cpad + 1, :], in_=negxx[:])

            # --- main chunk loop, split so chunk 0's top-k starts before the
            # at-table phase (which only the gathers depend on) ---
            def emit_front(i):
                # gram s[n, m] for n in chunk i : two psum halves [128, 1024]
                ps_h = []
                for h in range(2):
                    ph = pp_s.tile([P, 1024], F32, space="PSUM")
                    for q in range(2):
                        sl = slice((2 * h + q) * 512, (2 * h + q + 1) * 512)
                        if C <= 64:
                            nc.tensor.matmul(out=ph[:, q * 512:(q + 1) * 512],
                                             lhsT=gl[:, i * P:(i + 1) * P], rhs=gr[:, sl],
                                             start=True, stop=True)
                        else:
                            nc.tensor.matmul(out=ph[:, q * 512:(q + 1) * 512],
                                             lhsT=xb[:, i * P:(i + 1) * P], rhs=x2[:, sl],
                                             start=True, stop=False)
                            nc.tensor.matmul(out=ph[:, q * 512:(q + 1) * 512],
                                             lhsT=ones_row[:, :P],
                                             rhs=negxx[:, sl],
                                             start=False, stop=True)
                    ps_h.append(ph)

                # s -> SBUF (full row needed by the match-op scans)
                s_sb = tkpool.tile([P, N], F32, tag="s_sb")
                for h in range(2):
                    nc.scalar.copy(out=s_sb[:, h * 1024:(h + 1) * 1024], in_=ps_h[h][:])

                # exact top-20 of each row: 3 rounds of (max8, max_index,
                # in-place match_replace); s_sb is consumed
                v8 = tkpool.tile([P, 8], F32, tag="v8")
                i24 = tkpool.tile([P, 24], U16, tag="i24")
                nc.vector.max(out=v8[:], in_=s_sb[:])
                nc.vector.max_index(out=i24[:, 0:8], in_max=v8[:], in_values=s_sb[:])
                # per-round u32 offset copies: round r's gathers can stream
                # while rounds r+1.. still scan (shortens the chunk tail that
                # gates each block boundary)
                nbr_a = tkpool.tile([P, 7], U32, tag="nbr_a")
                nc.vector.tensor_copy(out=nbr_a[:], in_=i24[:, 1:8])
                nc.vector.match_replace(out=s_sb[:], in_to_replace=v8[:], in_values=s_sb[:],
                                        imm_value=NEG)
                nc.vector.max(out=v8[:], in_=s_sb[:])
                nc.vector.max_index(out=i24[:, 8:16], in_max=v8[:], in_values=s_sb[:])
                nbr_b = tkpool.tile([P, 8], U32, tag="nbr_b")
                nc.vector.tensor_copy(out=nbr_b[:], in_=i24[:, 8:16])
                nc.vector.match_replace(out=s_sb[:], in_to_replace=v8[:], in_values=s_sb[:],
                                        imm_value=NEG)
                nc.vector.max(out=v8[:], in_=s_sb[:])
                nc.vector.max_index(out=i24[:, 16:24], in_max=v8[:], in_values=s_sb[:])
                nbr_c = tkpool.tile([P, 4], U32, tag="nbr_c")
                nc.vector.tensor_copy(out=nbr_c[:], in_=i24[:, 16:20])
                return (i24, nbr_a, nbr_b, nbr_c)

            def emit_back(i, idxs):
                i24, nbr_a, nbr_b, nbr_c = idxs
                # gather neighbor A^T rows and reduce max over k.
                # k=0 is always the point itself (s[n,n] is the row max; for an
                # exact-duplicate point the A rows are identical), so it comes
                # from one contiguous DMA instead of an indirect gather.
                gath = gpool.tile([P, KNN, O], at_dt, tag="gath")
                nc.sync.dma_start(out=gath[:, 0, :],
                                  in_=at_dram[bi][i * P:(i + 1) * P, :])
                for j in range(1, KNN):
                    if j < 8:
                        off = nbr_a[:, j - 1:j]
                    elif j < 16:
                        off = nbr_b[:, j - 8:j - 7]
                    else:
                        off = nbr_c[:, j - 16:j - 15]
                    nc.gpsimd.indirect_dma_start(
                        out=gath[:, j, :], out_offset=None, in_=at_dram[bi][:],
                        in_offset=IndirectOffsetOnAxis(ap=off, axis=0))

                # k-max tree, in place: 20 -> 10 -> 5 -> (2,2,1) -> 1
                # (contiguous slices; the bf16 block gets the 2x DVE mode)
                g = gath[:]
                nc.vector.tensor_tensor(out=g[:, 0:10, :], in0=g[:, 0:10, :],
                                        in1=g[:, 10:20, :], op=ALU.max)
                nc.vector.tensor_tensor(out=g[:, 0:5, :], in0=g[:, 0:5, :],
                                        in1=g[:, 5:10, :], op=ALU.max)
                nc.vector.tensor_tensor(out=g[:, 0:2, :], in0=g[:, 0:2, :],
                                        in1=g[:, 2:4, :], op=ALU.max)
                nc.vector.tensor_tensor(out=g[:, 0:1, :], in0=g[:, 0:1, :],
                                        in1=g[:, 1:2, :], op=ALU.max)
                nc.vector.tensor_tensor(out=g[:, 0:1, :], in0=g[:, 0:1, :],
                                        in1=g[:, 4:5, :], op=ALU.max)

                # B^T chunk, add, leaky relu (fused max(v, 0.2v))
                bt = pp_m.tile([P, O], F32, space="PSUM", tag="mm")
                nc.tensor.matmul(out=bt[:], lhsT=xb[:, i * P:(i + 1) * P], rhs=wdT[bi][:],
                                 start=True, stop=True)
                xt = gpool.tile([P, O], F32, tag="xt")
                nc.vector.tensor_add(out=xt[:], in0=g[:, 0, :], in1=bt[:])
                nc.vector.scalar_tensor_tensor(out=xt[:], in0=xt[:], scalar=SLOPE,
                                               in1=xt[:], op0=ALU.mult, op1=ALU.max)

                # transpose back to [O, chunk] into the x_cat tiles
                dsts = block_out_dst(bi)
                for q in range((O + P - 1) // P):
                    osz = min(P, O - q * P)
                    tp = pp_t.tile([P, P], F32, space="PSUM")
                    nc.tensor.transpose(out=tp[:osz, :], in_=xt[:, q * P:q * P + osz],
                                        identity=ident[:])
                    nc.scalar.copy(out=dsts[q][0:osz, i * P:(i + 1) * P], in_=tp[:osz, :])

                # overlap the global conv with block 4: quarter i//4 of the
                # xcat columns is final after chunk 4*(i//4)+3; drain one
                # pending (nq, oc) unit into the PE slack of later chunks
                if bi == 3:
                    if i % 4 == 3:
                        gconv_pending.extend((i // 4, oc) for oc in range(8))
                    if i >= 4:
                        for _ in range(2):
                            if gconv_pending:
                                emit_gconv_unit(*gconv_pending.pop(0))

            i24_0 = emit_front(0)
            i24_1 = emit_front(1)
            emit_at_table()
            emit_back(0, i24_0)
            emit_back(1, i24_1)
            for i in range(2, NCHUNK):
                i24_i = emit_front(i)
                emit_back(i, i24_i)

            if bi == 1:
                # assemble c12 = [x1; x2] for the global conv
                nc.sync.dma_start(out=c12[0:64, :], in_=x1t[:])
                nc.sync.dma_start(out=c12[64:128, :], in_=x2t[:])

        # ================= global conv + max (mostly interleaved above) ======
        while gconv_pending:
            emit_gconv_unit(*gconv_pending.pop(0))
        for oc in range(8):
            red1 = spool.tile([P, 1], F32, tag="red1")
            nc.vector.tensor_reduce(out=red1[:], in_=redv_all[:, oc * 4:(oc + 1) * 4],
                                    axis=mybir.AxisListType.X, op=ALU.max)
            nc.vector.scalar_tensor_tensor(out=red1[:], in0=red1[:], scalar=SLOPE,
                                           in1=red1[:], op0=ALU.mult, op1=ALU.max)
            nc.sync.dma_start(out=out_d[oc * P:(oc + 1) * P, :], in_=red1[:])

    nc.compile()
    return nc


_NC_CACHE = None


def _get_nc():
    global _NC_CACHE
    if _NC_CACHE is None:
        _NC_CACHE = build_nc()
    return _NC_CACHE


def host_weights(W1, W2, W3, W4, Wg):
    ws = {}
    for bi, (wm, (c, o)) in enumerate(zip([W1, W2, W3, W4], BLOCKS)):
        wm = np.asarray(wm, dtype=np.float32)
        wn = wm[:, :c]
        wd = wm[:, c:] - wn
        ws[f"wnt{bi}"] = np.ascontiguousarray(wn.T)
        ws[f"wdt{bi}"] = np.ascontiguousarray(wd.T)
    ws["wgt"] = np.ascontiguousarray(np.asarray(Wg, dtype=np.float32).T)
    ws["ident"] = np.eye(P, dtype=np.float32)
    return ws


def kernel(x, W1, W2, W3, W4, Wg):
    from concourse.bass_utils import run_bass_kernel_spmd

    nc = _get_nc()
    x = np.asarray(x, dtype=np.float32)
    ws = host_weights(W1, W2, W3, W4, Wg)
    in_maps = [{"x": np.ascontiguousarray(x[b]), **ws} for b in range(B)]
    res = run_bass_kernel_spmd(nc, in_maps, core_ids=list(range(B)))
    outs = res.results if hasattr(res, "results") else res
    return np.stack([outs[b]["out"].reshape(1024) for b in range(B)], axis=0)


# revision 30
# speedup vs baseline: 1.3080x; 1.3080x over previous
"""DGCNN forward on 8 Trainium2 NeuronCores, data-parallel over batch.

Per core (one point cloud, x [3, 2048]):
  4 edge-conv blocks, each:
    s[n,m] = 2*x_n.x_m - |x_m|^2            (augmented fp32 PE matmul; row-constant
                                             -|x_n|^2 dropped: rank-invariant per row)
    top-20 of each s row:  3 rounds of (max8, max_index, match_replace)
    x' = lrelu(max_k A[:, idx_k] + B)       (A = Wn@x, B = (Wc-Wn)@x; edge features
                                             never materialized: conv is linear and
                                             lrelu/max commute)
    The k=0 neighbor is always the point itself (s[n,n] is the row max), so its
    A-row comes from one contiguous DMA; only k=1..19 use indirect gathers.
    Block 4's A-table is bf16 (its output feeds only the global conv, no further
    knn), halving its gather traffic and enabling the 2x DVE mode for its k-max.
  then g = lrelu(Wg @ concat(x1..x4)), out = max_n g.

knn grams and blocks 1-3 stay exact fp32 (noise there corrupts the knn sets).
"""

import numpy as np
from contextlib import ExitStack

import concourse.bass as bass
import concourse.bacc as bacc
import concourse.mybir as mybir
from concourse.bass import IndirectOffsetOnAxis
from concourse.tile import TileContext

F32 = mybir.dt.float32
BF16 = mybir.dt.bfloat16
U16 = mybir.dt.uint16
U32 = mybir.dt.uint32

B, N, KNN, P = 8, 2048, 20, 128
NCHUNK = N // P            # 16
NEG = -3.0e38
SLOPE = 0.2
BLOCKS = [(3, 64), (64, 64), (64, 128), (128, 256)]
ACT = mybir.ActivationFunctionType
ALU = mybir.AluOpType


def build_nc():
    nc = bacc.Bacc("TRN2", target_bir_lowering=False)

    x_in = nc.dram_tensor("x", [3, N], F32, kind="ExternalInput")
    w_in = {}
    for bi, (c, o) in enumerate(BLOCKS):
        w_in[f"wnt{bi}"] = nc.dram_tensor(f"wnt{bi}", [c, o], F32, kind="ExternalInput")
        w_in[f"wdt{bi}"] = nc.dram_tensor(f"wdt{bi}", [c, o], F32, kind="ExternalInput")
    w_in["wgt"] = nc.dram_tensor("wgt", [512, 1024], F32, kind="ExternalInput")
    id_in = nc.dram_tensor("ident", [P, P], F32, kind="ExternalInput")
    out_d = nc.dram_tensor("out", [1024, 1], F32, kind="ExternalOutput")

    # DRAM scratch: per-block A^T feature tables
    at_dram = [
        nc.dram_tensor(f"at{bi}_scratch", [N, o], BF16 if bi == 3 else F32,
                       kind="Internal")
        for bi, (_, o) in enumerate(BLOCKS)
    ]

    with TileContext(nc) as tc, ExitStack() as ctx:
        ep = ctx.enter_context            # shorthand
        const = ep(tc.tile_pool(name="const", bufs=1))
        wpool = ep(tc.tile_pool(name="weights", bufs=1))
        xpool = ep(tc.tile_pool(name="xtiles", bufs=1))
        bpool = ep(tc.tile_pool(name="blockp", bufs=1))
        spool = ep(tc.tile_pool(name="sbuf", bufs=2))
        tkpool = ep(tc.tile_pool(name="topk", bufs=2))
        gpool = ep(tc.tile_pool(name="gather", bufs=2))
        perb = ep(tc.tile_pool(name="perblock", bufs=1))
        pp_s = ep(tc.tile_pool(name="ps_s", bufs=2, space="PSUM"))
        pp_m = ep(tc.tile_pool(name="ps_m", bufs=2, space="PSUM"))
        pp_t = ep(tc.tile_pool(name="ps_t", bufs=2, space="PSUM"))

        # block input x0 — first DMA: everything in block 1 hangs off it
        x0 = xpool.tile([3, N], F32)
        nc.sync.dma_start(out=x0[:], in_=x_in[:])

        # ---- constants ----
        ident = const.tile([P, P], F32)
        nc.sync.dma_start(out=ident[:], in_=id_in[:])
        ones_col = const.tile([P, 1], F32)
        nc.vector.memset(ones_col[:], 1.0)
        ones_row = const.tile([1, P], F32)
        nc.vector.memset(ones_row[:], 1.0)

        # ---- load pre-transposed weights (host supplies WnT/WdT/WgT) ----
        wnT, wdT = [], []
        for bi, (c, o) in enumerate(BLOCKS):
            wn = wpool.tile([c, o], F32, tag=f"wn{bi}")
            nc.sync.dma_start(out=wn[:], in_=w_in[f"wnt{bi}"][:])
            wd = wpool.tile([c, o], F32, tag=f"wd{bi}")
            nc.sync.dma_start(out=wd[:], in_=w_in[f"wdt{bi}"][:])
            wnT.append(wn)
            wdT.append(wd)

        # WgT loads are deferred into block 2 (first needed by block 4's gconv)
        wgT = [wpool.tile([P, 1024], F32, tag=f"wg{k}", name=f"wgT{k}") for k in range(4)]

        # ---- x_cat tiles (c12 assembled from x1t/x2t before the global conv) ----
        c12 = xpool.tile([P, N], F32)
        c3 = xpool.tile([P, N], F32)
        c4a = xpool.tile([P, N], F32)
        c4b = xpool.tile([P, N], F32)
        x1t = xpool.tile([64, N], F32)
        x2t = xpool.tile([64, N], F32)

        # global-conv partial maxima, filled as block 4 quarters complete
        redv_all = perb.tile([P, 32], F32, name="redv_all")

        def emit_gconv_unit(nq, oc):
            xcat = [c12, c3, c4a, c4b]
            pg = pp_s.tile([P, 512], F32, space="PSUM", tag="ph")
            for k in range(4):
                nc.tensor.matmul(out=pg[:], lhsT=wgT[k][:, oc * P:(oc + 1) * P],
                                 rhs=xcat[k][:, nq * 512:(nq + 1) * 512],
                                 start=(k == 0), stop=(k == 3))
            nc.vector.tensor_reduce(out=redv_all[:, oc * 4 + nq:oc * 4 + nq + 1],
                                    in_=pg[:], axis=mybir.AxisListType.X,
                                    op=ALU.max)

        def block_input(bi):
            return [x0[:], x1t[:], x2t[:], c3[:]][bi]

        def block_out_dst(bi):
            return [[x1t], [x2t], [c3], [c4a, c4b]][bi]

        gconv_pending = []

        # ================= edge blocks =================
        for bi, (C, O) in enumerate(BLOCKS):
            xb = block_input(bi)
            at_dt = BF16 if bi == 3 else F32

            if bi == 1:
                for k in range(4):
                    nc.sync.dma_start(out=wgT[k][:],
                                      in_=w_in["wgt"][k * P:(k + 1) * P, :])

            # --- per-block prep: xsq first (it feeds the negxx matmuls) ---
            xsq = bpool.tile([C, N], F32, tag="xsq")
            nc.scalar.activation(out=xsq[:], in_=xb, func=ACT.Square)
            x2 = bpool.tile([C, N], F32, tag="x2")
            nc.scalar.activation(out=x2[:], in_=xb, func=ACT.Copy, scale=2.0)
            negxx = bpool.tile([1, N], F32, tag="negxx")
            for q in range(4):
                mq = pp_m.tile([1, 512], F32, space="PSUM", tag="mm")
                nc.tensor.matmul(out=mq[:], lhsT=ones_col[:C, :], rhs=xsq[:, q * 512:(q + 1) * 512],
                                 start=True, stop=True)
                nc.scalar.activation(out=negxx[:, q * 512:(q + 1) * 512], in_=mq[:],
                                     func=ACT.Copy, scale=-1.0)

            def emit_at_table():
                # A^T rows to DRAM: at[n, :] = x_n . WnT  (chunk-wise).
                # Emitted after the first grams: it only gates the gathers.
                for i in range(NCHUNK):
                    ap_ = pp_m.tile([P, O], F32, space="PSUM", tag="mm")
                    nc.tensor.matmul(out=ap_[:], lhsT=xb[:, i * P:(i + 1) * P],
                                     rhs=wnT[bi][:], start=True, stop=True)
                    at_sb = spool.tile([P, O], at_dt, tag="at_sb")
                    nc.scalar.copy(out=at_sb[:], in_=ap_[:])
                    nc.sync.dma_start(out=at_dram[bi][i * P:(i + 1) * P, :], in_=at_sb[:])

            # augmented gram lhs/rhs for C <= 64 (single fused matmul); block4 separate
            if C <= 64:
                # augmented row must start at a 32-aligned partition; pad with zeros
                cpad = ((C + 31) // 32) * 32
                gl = bpool.tile([cpad + 1, N], F32, tag="gramL")
                gr = bpool.tile([cpad + 1, N], F32, tag="gramR")
                if cpad != C:
                    nc.vector.memset(gl[:], 0.0)
                    nc.vector.memset(gr[:], 0.0)
                nc.scalar.copy(out=gl[:C, :], in_=xb)
                nc.vector.memset(gl[cpad:cpad + 1, :], 1.0)
                nc.vector.tensor_copy(out=gr[:C, :], in_=x2[:])
                # engines are lane-aligned: partition 0 -> cpad needs a DMA
                nc.sync.dma_start(out=gr[cpad:cpad + 1, :], in_=negxx[:])

            # --- main chunk loop, split so chunk 0's top-k starts before the
            # at-table phase (which only the gathers depend on) ---
            def emit_front(i):
                # gram s[n, m] for n in chunk i : two psum halves [128, 1024]
                ps_h = []
                for h in range(2):
                    ph = pp_s.tile([P, 1024], F32, space="PSUM")
                    for q in range(2):
                        sl = slice((2 * h + q) * 512, (2 * h + q + 1) * 512)
                        if C <= 64:
                            nc.tensor.matmul(out=ph[:, q * 512:(q + 1) * 512],
                                             lhsT=gl[:, i * P:(i + 1) * P], rhs=gr[:, sl],
                                             start=True, stop=True)
                        else:
                            nc.tensor.matmul(out=ph[:, q * 512:(q + 1) * 512],
                                             lhsT=xb[:, i * P:(i + 1) * P], rhs=x2[:, sl],
                                             start=True, stop=False)
                            nc.tensor.matmul(out=ph[:, q * 512:(q + 1) * 512],
                                             lhsT=ones_row[:, :P],
                                             rhs=negxx[:, sl],
                                             start=False, stop=True)
                    ps_h.append(ph)

                # s -> SBUF (full row needed by the match-op scans)
                s_sb = tkpool.tile([P, N], F32, tag="s_sb")
                for h in range(2):
                    nc.scalar.copy(out=s_sb[:, h * 1024:(h + 1) * 1024], in_=ps_h[h][:])

                # exact top-20 of each row: 3 rounds of (max8, max_index,
                # in-place match_replace); s_sb is consumed
                # max_index writes u32 directly: each round's gathers stream
                # off its output with no staging copy (shortens the chunk tail
                # that gates each block boundary)
                v8 = tkpool.tile([P, 8], F32, tag="v8")
                nbr_a = tkpool.tile([P, 8], U32, tag="nbr_a")
                nbr_b = tkpool.tile([P, 8], U32, tag="nbr_b")
                nbr_c = tkpool.tile([P, 8], U32, tag="nbr_c")
                nc.vector.max(out=v8[:], in_=s_sb[:])
                nc.vector.max_index(out=nbr_a[:], in_max=v8[:], in_values=s_sb[:])
                nc.vector.match_replace(out=s_sb[:], in_to_replace=v8[:], in_values=s_sb[:],
                                        imm_value=NEG)
                nc.vector.max(out=v8[:], in_=s_sb[:])
                nc.vector.max_index(out=nbr_b[:], in_max=v8[:], in_values=s_sb[:])
                nc.vector.match_replace(out=s_sb[:], in_to_replace=v8[:], in_values=s_sb[:],
                                        imm_value=NEG)
                nc.vector.max(out=v8[:], in_=s_sb[:])
                nc.vector.max_index(out=nbr_c[:], in_max=v8[:], in_values=s_sb[:])
                return (nbr_a, nbr_b, nbr_c)

            def emit_back(i, idxs):
                nbr_a, nbr_b, nbr_c = idxs
                # gather neighbor A^T rows and reduce max over k.
                # k=0 is always the point itself (s[n,n] is the row max; for an
                # exact-duplicate point the A rows are identical), so it comes
                # from one contiguous DMA instead of an indirect gather.
                gath = gpool.tile([P, KNN, O], at_dt, tag="gath")
                nc.sync.dma_start(out=gath[:, 0, :],
                                  in_=at_dram[bi][i * P:(i + 1) * P, :])
                for j in range(1, KNN):
                    if j < 8:
                        off = nbr_a[:, j:j + 1]
                    elif j < 16:
                        off = nbr_b[:, j - 8:j - 7]
                    else:
                        off = nbr_c[:, j - 16:j - 15]
                    nc.gpsimd.indirect_dma_start(
                        out=gath[:, j, :], out_offset=None, in_=at_dram[bi][:],
                        in_offset=IndirectOffsetOnAxis(ap=off, axis=0))

                # k-max tree, in place: 20 -> 10 -> 5 -> (2,2,1) -> 1
                # (contiguous slices; the bf16 block gets the 2x DVE mode)
                g = gath[:]
                nc.vector.tensor_tensor(out=g[:, 0:10, :], in0=g[:, 0:10, :],
                                        in1=g[:, 10:20, :], op=ALU.max)
                nc.vector.tensor_tensor(out=g[:, 0:5, :], in0=g[:, 0:5, :],
                                        in1=g[:, 5:10, :], op=ALU.max)
                nc.vector.tensor_tensor(out=g[:, 0:2, :], in0=g[:, 0:2, :],
                                        in1=g[:, 2:4, :], op=ALU.max)
                nc.vector.tensor_tensor(out=g[:, 0:1, :], in0=g[:, 0:1, :],
                                        in1=g[:, 1:2, :], op=ALU.max)
                nc.vector.tensor_tensor(out=g[:, 0:1, :], in0=g[:, 0:1, :],
                                        in1=g[:, 4:5, :], op=ALU.max)

                # B^T chunk, add, leaky relu (fused max(v, 0.2v))
                bt = pp_m.tile([P, O], F32, space="PSUM", tag="mm")
                nc.tensor.matmul(out=bt[:], lhsT=xb[:, i * P:(i + 1) * P], rhs=wdT[bi][:],
                                 start=True, stop=True)
                xt = gpool.tile([P, O], F32, tag="xt")
                nc.vector.tensor_add(out=xt[:], in0=g[:, 0, :], in1=bt[:])
                nc.vector.scalar_tensor_tensor(out=xt[:], in0=xt[:], scalar=SLOPE,
                                               in1=xt[:], op0=ALU.mult, op1=ALU.max)

                # transpose back to [O, chunk] into the x_cat tiles
                dsts = block_out_dst(bi)
                for q in range((O + P - 1) // P):
                    osz = min(P, O - q * P)
                    tp = pp_t.tile([P, P], F32, space="PSUM")
                    nc.tensor.transpose(out=tp[:osz, :], in_=xt[:, q * P:q * P + osz],
                                        identity=ident[:])
                    nc.scalar.copy(out=dsts[q][0:osz, i * P:(i + 1) * P], in_=tp[:osz, :])

                # overlap the global conv with block 4: quarter i//4 of the
                # xcat columns is final after chunk 4*(i//4)+3; drain one
                # pending (nq, oc) unit into the PE slack of later chunks
                if bi == 3:
                    if i % 4 == 3:
                        gconv_pending.extend((i // 4, oc) for oc in range(8))
                    if i >= 4:
                        for _ in range(2):
                            if gconv_pending:
                                emit_gconv_unit(*gconv_pending.pop(0))

            i24_0 = emit_front(0)
            i24_1 = emit_front(1)
            emit_at_table()
            emit_back(0, i24_0)
            emit_back(1, i24_1)
            for i in range(2, NCHUNK):
                i24_i = emit_front(i)
                emit_back(i, i24_i)

            if bi == 1:
                # assemble c12 = [x1; x2] for the global conv
                nc.sync.dma_start(out=c12[0:64, :], in_=x1t[:])
                nc.sync.dma_start(out=c12[64:128, :], in_=x2t[:])

        # ================= global conv + max (mostly interleaved above) ======
        while gconv_pending:
            emit_gconv_unit(*gconv_pending.pop(0))
        for oc in range(8):
            red1 = spool.tile([P, 1], F32, tag="red1")
            nc.vector.tensor_reduce(out=red1[:], in_=redv_all[:, oc * 4:(oc + 1) * 4],
                                    axis=mybir.AxisListType.X, op=ALU.max)
            nc.vector.scalar_tensor_tensor(out=red1[:], in0=red1[:], scalar=SLOPE,
                                           in1=red1[:], op0=ALU.mult, op1=ALU.max)
            nc.sync.dma_start(out=out_d[oc * P:(oc + 1) * P, :], in_=red1[:])

    nc.compile()
    return nc


_NC_CACHE = None


def _get_nc():
    global _NC_CACHE
    if _NC_CACHE is None:
        _NC_CACHE = build_nc()
    return _NC_CACHE


def host_weights(W1, W2, W3, W4, Wg):
    ws = {}
    for bi, (wm, (c, o)) in enumerate(zip([W1, W2, W3, W4], BLOCKS)):
        wm = np.asarray(wm, dtype=np.float32)
        wn = wm[:, :c]
        wd = wm[:, c:] - wn
        ws[f"wnt{bi}"] = np.ascontiguousarray(wn.T)
        ws[f"wdt{bi}"] = np.ascontiguousarray(wd.T)
    ws["wgt"] = np.ascontiguousarray(np.asarray(Wg, dtype=np.float32).T)
    ws["ident"] = np.eye(P, dtype=np.float32)
    return ws


def kernel(x, W1, W2, W3, W4, Wg):
    from concourse.bass_utils import run_bass_kernel_spmd

    nc = _get_nc()
    x = np.asarray(x, dtype=np.float32)
    ws = host_weights(W1, W2, W3, W4, Wg)
    in_maps = [{"x": np.ascontiguousarray(x[b]), **ws} for b in range(B)]
    res = run_bass_kernel_spmd(nc, in_maps, core_ids=list(range(B)))
    outs = res.results if hasattr(res, "results") else res
    return np.stack([outs[b]["out"].reshape(1024) for b in range(B)], axis=0)


# revision 31
# speedup vs baseline: 2.8548x; 2.1827x over previous
"""DGCNN forward on 8 Trainium2 NeuronCores, data-parallel over batch.

Per core (one point cloud, x [3, 2048]):
  4 edge-conv blocks, each:
    s[n,m] = 2*x_n.x_m - |x_m|^2            (augmented fp32 PE matmul; row-constant
                                             -|x_n|^2 dropped: rank-invariant per row)
    top-20 of each s row:  3 rounds of (max8, max_index, match_replace)
    x' = lrelu(max_k A[:, idx_k] + B)       (A = Wn@x, B = (Wc-Wn)@x; edge features
                                             never materialized: conv is linear and
                                             lrelu/max commute)
    The k=0 neighbor is always the point itself (s[n,n] is the row max), so its
    A-row comes from one contiguous DMA; only k=1..19 use indirect gathers.
    Block 4's A-table is bf16 (its output feeds only the global conv, no further
    knn), halving its gather traffic and enabling the 2x DVE mode for its k-max.
  then g = lrelu(Wg @ concat(x1..x4)), out = max_n g.

knn grams and blocks 1-3 stay exact fp32 (noise there corrupts the knn sets).
"""

import numpy as np
from contextlib import ExitStack

import concourse.bass as bass
import concourse.bacc as bacc
import concourse.mybir as mybir
from concourse.bass import IndirectOffsetOnAxis
from concourse.tile import TileContext

F32 = mybir.dt.float32
BF16 = mybir.dt.bfloat16
U16 = mybir.dt.uint16
U32 = mybir.dt.uint32

B, N, KNN, P = 8, 2048, 20, 128
NCHUNK = N // P            # 16
NEG = -3.0e38
SLOPE = 0.2
BLOCKS = [(3, 64), (64, 64), (64, 128), (128, 256)]
ACT = mybir.ActivationFunctionType
ALU = mybir.AluOpType


def build_nc():
    nc = bacc.Bacc("TRN2", target_bir_lowering=False)

    x_in = nc.dram_tensor("x", [3, N], F32, kind="ExternalInput")
    w_in = {}
    for bi, (c, o) in enumerate(BLOCKS):
        w_in[f"wnt{bi}"] = nc.dram_tensor(f"wnt{bi}", [c, o], F32, kind="ExternalInput")
        w_in[f"wdt{bi}"] = nc.dram_tensor(f"wdt{bi}", [c, o], F32, kind="ExternalInput")
    w_in["wgt"] = nc.dram_tensor("wgt", [512, 1024], F32, kind="ExternalInput")
    id_in = nc.dram_tensor("ident", [P, P], F32, kind="ExternalInput")
    out_d = nc.dram_tensor("out", [1024, 1], F32, kind="ExternalOutput")

    # DRAM scratch: per-block A^T feature tables
    at_dram = [
        nc.dram_tensor(f"at{bi}_scratch", [N, o], BF16 if bi == 3 else F32,
                       kind="Internal")
        for bi, (_, o) in enumerate(BLOCKS)
    ]

    with TileContext(nc) as tc, ExitStack() as ctx:
        ep = ctx.enter_context            # shorthand
        const = ep(tc.tile_pool(name="const", bufs=1))
        wpool = ep(tc.tile_pool(name="weights", bufs=1))
        xpool = ep(tc.tile_pool(name="xtiles", bufs=1))
        bpool = ep(tc.tile_pool(name="blockp", bufs=1))
        spool = ep(tc.tile_pool(name="sbuf", bufs=2))
        tkpool = ep(tc.tile_pool(name="topk", bufs=2))
        gpool = ep(tc.tile_pool(name="gather", bufs=2))
        perb = ep(tc.tile_pool(name="perblock", bufs=1))
        pp_s = ep(tc.tile_pool(name="ps_s", bufs=2, space="PSUM"))
        pp_m = ep(tc.tile_pool(name="ps_m", bufs=2, space="PSUM"))
        pp_t = ep(tc.tile_pool(name="ps_t", bufs=2, space="PSUM"))

        # block input x0 — first DMA: everything in block 1 hangs off it
        x0 = xpool.tile([3, N], F32)
        nc.sync.dma_start(out=x0[:], in_=x_in[:])

        # ---- constants ----
        ident = const.tile([P, P], F32)
        nc.sync.dma_start(out=ident[:], in_=id_in[:])
        ones_col = const.tile([P, 1], F32)
        nc.vector.memset(ones_col[:], 1.0)
        ones_row = const.tile([1, P], F32)
        nc.vector.memset(ones_row[:], 1.0)

        # ---- load pre-transposed weights (host supplies WnT/WdT/WgT) ----
        wnT, wdT = [], []
        for bi, (c, o) in enumerate(BLOCKS):
            wn = wpool.tile([c, o], F32, tag=f"wn{bi}")
            nc.sync.dma_start(out=wn[:], in_=w_in[f"wnt{bi}"][:])
            wd = wpool.tile([c, o], F32, tag=f"wd{bi}")
            nc.sync.dma_start(out=wd[:], in_=w_in[f"wdt{bi}"][:])
            wnT.append(wn)
            wdT.append(wd)

        # WgT loads are deferred into block 2 (first needed by block 4's gconv)
        wgT = [wpool.tile([P, 1024], F32, tag=f"wg{k}", name=f"wgT{k}") for k in range(4)]

        # ---- x_cat tiles (c12 assembled from x1t/x2t before the global conv) ----
        c12 = xpool.tile([P, N], F32)
        c3 = xpool.tile([P, N], F32)
        c4a = xpool.tile([P, N], F32)
        c4b = xpool.tile([P, N], F32)
        x1t = xpool.tile([64, N], F32)
        x2t = xpool.tile([64, N], F32)

        # global-conv partial maxima, filled as block 4 quarters complete
        redv_all = perb.tile([P, 32], F32, name="redv_all")

        def emit_gconv_unit(nq, oc):
            xcat = [c12, c3, c4a, c4b]
            pg = pp_s.tile([P, 512], F32, space="PSUM", tag="ph")
            for k in range(4):
                nc.tensor.matmul(out=pg[:], lhsT=wgT[k][:, oc * P:(oc + 1) * P],
                                 rhs=xcat[k][:, nq * 512:(nq + 1) * 512],
                                 start=(k == 0), stop=(k == 3))
            nc.vector.tensor_reduce(out=redv_all[:, oc * 4 + nq:oc * 4 + nq + 1],
                                    in_=pg[:], axis=mybir.AxisListType.X,
                                    op=ALU.max)

        def block_input(bi):
            return [x0[:], x1t[:], x2t[:], c3[:]][bi]

        def block_out_dst(bi):
            return [[x1t], [x2t], [c3], [c4a, c4b]][bi]

        gconv_pending = []

        # ================= edge blocks =================
        for bi, (C, O) in enumerate(BLOCKS):
            xb = block_input(bi)
            at_dt = BF16 if bi == 3 else F32

            if bi == 1:
                for k in range(4):
                    nc.sync.dma_start(out=wgT[k][:],
                                      in_=w_in["wgt"][k * P:(k + 1) * P, :])

            # --- per-block prep: xsq first (it feeds the negxx matmuls) ---
            xsq = bpool.tile([C, N], F32, tag="xsq")
            nc.scalar.activation(out=xsq[:], in_=xb, func=ACT.Square)
            x2 = bpool.tile([C, N], F32, tag="x2")
            nc.scalar.activation(out=x2[:], in_=xb, func=ACT.Copy, scale=2.0)
            negxx = bpool.tile([1, N], F32, tag="negxx")
            for q in range(4):
                mq = pp_m.tile([1, 512], F32, space="PSUM", tag="mm")
                nc.tensor.matmul(out=mq[:], lhsT=ones_col[:C, :], rhs=xsq[:, q * 512:(q + 1) * 512],
                                 start=True, stop=True)
                nc.scalar.activation(out=negxx[:, q * 512:(q + 1) * 512], in_=mq[:],
                                     func=ACT.Copy, scale=-1.0)

            def emit_at_table():
                # A^T rows to DRAM: at[n, :] = x_n . WnT  (chunk-wise).
                # Emitted after the first grams: it only gates the gathers.
                for i in range(NCHUNK):
                    ap_ = pp_m.tile([P, O], F32, space="PSUM", tag="mm")
                    nc.tensor.matmul(out=ap_[:], lhsT=xb[:, i * P:(i + 1) * P],
                                     rhs=wnT[bi][:], start=True, stop=True)
                    at_sb = spool.tile([P, O], at_dt, tag="at_sb")
                    nc.scalar.copy(out=at_sb[:], in_=ap_[:])
                    nc.sync.dma_start(out=at_dram[bi][i * P:(i + 1) * P, :], in_=at_sb[:])

            # augmented gram lhs/rhs for C <= 64 (single fused matmul); block4 separate
            if C <= 64:
                # augmented row must start at a 32-aligned partition; pad with zeros
                cpad = ((C + 31) // 32) * 32
                gl = bpool.tile([cpad + 1, N], F32, tag="gramL")
                gr = bpool.tile([cpad + 1, N], F32, tag="gramR")
                if cpad != C:
                    nc.vector.memset(gl[:], 0.0)
                    nc.vector.memset(gr[:], 0.0)
                nc.scalar.copy(out=gl[:C, :], in_=xb)
                nc.vector.memset(gl[cpad:cpad + 1, :], 1.0)
                nc.vector.tensor_copy(out=gr[:C, :], in_=x2[:])
                # engines are lane-aligned: partition 0 -> cpad needs a DMA
                nc.sync.dma_start(out=gr[cpad:cpad + 1, :], in_=negxx[:])

            # --- main chunk loop, split so chunk 0's top-k starts before the
            # at-table phase (which only the gathers depend on) ---
            def emit_front(i):
                # gram s[n, m] for n in chunk i : two psum halves [128, 1024]
                ps_h = []
                for h in range(2):
                    ph = pp_s.tile([P, 1024], F32, space="PSUM")
                    for q in range(2):
                        sl = slice((2 * h + q) * 512, (2 * h + q + 1) * 512)
                        if C <= 64:
                            nc.tensor.matmul(out=ph[:, q * 512:(q + 1) * 512],
                                             lhsT=gl[:, i * P:(i + 1) * P], rhs=gr[:, sl],
                                             start=True, stop=True)
                        else:
                            nc.tensor.matmul(out=ph[:, q * 512:(q + 1) * 512],
                                             lhsT=xb[:, i * P:(i + 1) * P], rhs=x2[:, sl],
                                             start=True, stop=False)
                            nc.tensor.matmul(out=ph[:, q * 512:(q + 1) * 512],
                                             lhsT=ones_row[:, :P],
                                             rhs=negxx[:, sl],
                                             start=False, stop=True)
                    ps_h.append(ph)

                # s -> SBUF (full row needed by the match-op scans)
                s_sb = tkpool.tile([P, N], F32, tag="s_sb")
                for h in range(2):
                    nc.scalar.copy(out=s_sb[:, h * 1024:(h + 1) * 1024], in_=ps_h[h][:])

                # exact top-20 of each row: 3 rounds of (max8, max_index,
                # in-place match_replace); s_sb is consumed
                # max_index writes u32 directly: each round's gathers stream
                # off its output with no staging copy (shortens the chunk tail
                # that gates each block boundary)
                v8 = tkpool.tile([P, 8], F32, tag="v8")
                nbr_a = tkpool.tile([P, 8], U32, tag="nbr_a")
                nbr_b = tkpool.tile([P, 8], U32, tag="nbr_b")
                nbr_c = tkpool.tile([P, 8], U32, tag="nbr_c")
                nc.vector.max(out=v8[:], in_=s_sb[:])
                nc.vector.max_index(out=nbr_a[:], in_max=v8[:], in_values=s_sb[:])
                nc.vector.match_replace(out=s_sb[:], in_to_replace=v8[:], in_values=s_sb[:],
                                        imm_value=NEG)
                nc.vector.max(out=v8[:], in_=s_sb[:])
                nc.vector.max_index(out=nbr_b[:], in_max=v8[:], in_values=s_sb[:])
                nc.vector.match_replace(out=s_sb[:], in_to_replace=v8[:], in_values=s_sb[:],
                                        imm_value=NEG)
                nc.vector.max(out=v8[:], in_=s_sb[:])
                nc.vector.max_index(out=nbr_c[:], in_max=v8[:], in_values=s_sb[:])
                return (nbr_a, nbr_b, nbr_c)

            def emit_back(i, idxs):
                nbr_a, nbr_b, nbr_c = idxs
                # gather neighbor A^T rows and reduce max over k.
                # k=0 is always the point itself (s[n,n] is the row max; for an
                # exact-duplicate point the A rows are identical), so it comes
                # from one contiguous DMA instead of an indirect gather.
                gath = gpool.tile([P, KNN, O], at_dt, tag="gath")
                nc.sync.dma_start(out=gath[:, 0, :],
                                  in_=at_dram[bi][i * P:(i + 1) * P, :])
                for j in range(1, KNN):
                    if j < 8:
                        off = nbr_a[:, j:j + 1]
                    elif j < 16:
                        off = nbr_b[:, j - 8:j - 7]
                    else:
                        off = nbr_c[:, j - 16:j - 15]
                    nc.gpsimd.indirect_dma_start(
                        out=gath[:, j, :], out_offset=None, in_=at_dram[bi][:],
                        in_offset=IndirectOffsetOnAxis(ap=off, axis=0))

                # k-max tree, in place: 20 -> 10 -> 5 -> (2,2,1) -> 1
                # (contiguous slices; the bf16 block gets the 2x DVE mode)
                g = gath[:]
                nc.vector.tensor_tensor(out=g[:, 0:10, :], in0=g[:, 0:10, :],
                                        in1=g[:, 10:20, :], op=ALU.max)
                nc.vector.tensor_tensor(out=g[:, 0:5, :], in0=g[:, 0:5, :],
                                        in1=g[:, 5:10, :], op=ALU.max)
                nc.vector.tensor_tensor(out=g[:, 0:2, :], in0=g[:, 0:2, :],
                                        in1=g[:, 2:4, :], op=ALU.max)
                nc.vector.tensor_tensor(out=g[:, 0:1, :], in0=g[:, 0:1, :],
                                        in1=g[:, 1:2, :], op=ALU.max)
                nc.vector.tensor_tensor(out=g[:, 0:1, :], in0=g[:, 0:1, :],
                                        in1=g[:, 4:5, :], op=ALU.max)

                # B^T chunk, add, leaky relu (fused max(v, 0.2v))
                bt = pp_m.tile([P, O], F32, space="PSUM", tag="mm")
                nc.tensor.matmul(out=bt[:], lhsT=xb[:, i * P:(i + 1) * P], rhs=wdT[bi][:],
                                 start=True, stop=True)
                xt = gpool.tile([P, O], F32, tag="xt")
                nc.vector.tensor_add(out=xt[:], in0=g[:, 0, :], in1=bt[:])
                nc.vector.scalar_tensor_tensor(out=xt[:], in0=xt[:], scalar=SLOPE,
                                               in1=xt[:], op0=ALU.mult, op1=ALU.max)

                # transpose back to [O, chunk] into the x_cat tiles
                dsts = block_out_dst(bi)
                for q in range((O + P - 1) // P):
                    osz = min(P, O - q * P)
                    tp = pp_t.tile([P, P], F32, space="PSUM")
                    nc.tensor.transpose(out=tp[:osz, :], in_=xt[:, q * P:q * P + osz],
                                        identity=ident[:])
                    nc.scalar.copy(out=dsts[q][0:osz, i * P:(i + 1) * P], in_=tp[:osz, :])

                # overlap the global conv with block 4: quarter i//4 of the
                # xcat columns is final after chunk 4*(i//4)+3; drain one
                # pending (nq, oc) unit into the PE slack of later chunks
                if bi == 3:
                    if i % 4 == 3:
                        gconv_pending.extend((i // 4, oc) for oc in range(8))
                    if i >= 4:
                        for _ in range(2):
                            if gconv_pending:
                                emit_gconv_unit(*gconv_pending.pop(0))

            i24_0 = emit_front(0)
            i24_1 = emit_front(1)
            emit_at_table()
            emit_back(0, i24_0)
            emit_back(1, i24_1)
            for i in range(2, NCHUNK):
                i24_i = emit_front(i)
                emit_back(i, i24_i)

            if bi == 1:
                # assemble c12 = [x1; x2] for the global conv
                nc.sync.dma_start(out=c12[0:64, :], in_=x1t[:])
                nc.sync.dma_start(out=c12[64:128, :], in_=x2t[:])

        # ================= global conv + max (mostly interleaved above) ======
        while gconv_pending:
            emit_gconv_unit(*gconv_pending.pop(0))
        for oc in range(8):
            red1 = spool.tile([P, 1], F32, tag="red1")
            nc.vector.tensor_reduce(out=red1[:], in_=redv_all[:, oc * 4:(oc + 1) * 4],
                                    axis=mybir.AxisListType.X, op=ALU.max)
            nc.vector.scalar_tensor_tensor(out=red1[:], in0=red1[:], scalar=SLOPE,
                                           in1=red1[:], op0=ALU.mult, op1=ALU.max)
            nc.sync.dma_start(out=out_d[oc * P:(oc + 1) * P, :], in_=red1[:])

    nc.compile()
    return nc


_NC_CACHE = None


def _get_nc():
    global _NC_CACHE
    if _NC_CACHE is None:
        _NC_CACHE = build_nc()
    return _NC_CACHE


def host_weights(W1, W2, W3, W4, Wg):
    ws = {}
    for bi, (wm, (c, o)) in enumerate(zip([W1, W2, W3, W4], BLOCKS)):
        wm = np.asarray(wm, dtype=np.float32)
        wn = wm[:, :c]
        wd = wm[:, c:] - wn
        ws[f"wnt{bi}"] = np.ascontiguousarray(wn.T)
        ws[f"wdt{bi}"] = np.ascontiguousarray(wd.T)
    ws["wgt"] = np.ascontiguousarray(np.asarray(Wg, dtype=np.float32).T)
    ws["ident"] = np.eye(P, dtype=np.float32)
    return ws


_EXEC_CACHE = None


def _get_exec(nc):
    """Build the 8-core shard_map executable once (mirrors
    bass2jax.run_bass_via_pjrt) so repeat kernel() calls skip retracing."""
    global _EXEC_CACHE
    if _EXEC_CACHE is not None:
        return _EXEC_CACHE
    import jax
    from jax.sharding import Mesh, PartitionSpec
    from jax.experimental.shard_map import shard_map
    from concourse import bass2jax

    bass2jax.install_neuronx_cc_hook()
    assert nc.dbg_addr is None
    partition_name = nc.partition_id_tensor.name if nc.partition_id_tensor else None
    in_names, out_names, out_avals, zero_shapes = [], [], [], []
    for alloc in nc.m.functions[0].allocations:
        if not isinstance(alloc, mybir.MemoryLocationSet):
            continue
        name = alloc.memorylocations[0].name
        if alloc.kind == "ExternalInput":
            if name != partition_name:
                in_names.append(name)
        elif alloc.kind == "ExternalOutput":
            out_names.append(name)
            shape = tuple(alloc.tensor_shape)
            dtype = mybir.dt.np(alloc.dtype)
            out_avals.append(jax.core.ShapedArray(shape, dtype))
            zero_shapes.append((shape, dtype))
    n_params = len(in_names)
    n_outs = len(out_avals)
    all_names = in_names + out_names
    if partition_name is not None:
        all_names.append(partition_name)
    donate = tuple(range(n_params, n_params + n_outs))

    def _body(*args):
        operands = list(args)
        if partition_name is not None:
            operands.append(bass2jax.partition_id_tensor())
        outs = bass2jax._bass_exec_p.bind(
            *operands,
            out_avals=tuple(out_avals),
            in_names=tuple(all_names),
            out_names=tuple(out_names),
            lowering_input_output_aliases=(),
            sim_require_finite=True,
            sim_require_nnan=True,
            nc=nc,
        )
        return tuple(outs)

    devices = jax.devices()[:B]
    mesh = Mesh(np.asarray(devices), ("core",))
    sharded = jax.jit(
        shard_map(_body, mesh=mesh,
                  in_specs=(PartitionSpec("core"),) * (n_params + n_outs),
                  out_specs=(PartitionSpec("core"),) * n_outs,
                  check_rep=False),
        donate_argnums=donate, keep_unused=True,
    )
    _EXEC_CACHE = (sharded, in_names, out_names, out_avals, zero_shapes)
    return _EXEC_CACHE


def kernel(x, W1, W2, W3, W4, Wg):
    nc = _get_nc()
    sharded, in_names, out_names, out_avals, zero_shapes = _get_exec(nc)
    x = np.asarray(x, dtype=np.float32)
    ws = host_weights(W1, W2, W3, W4, Wg)
    in_maps = [{"x": np.ascontiguousarray(x[b]), **ws} for b in range(B)]
    concat_in = [
        np.concatenate([np.asarray(in_maps[c][name]) for c in range(B)], axis=0)
        for name in in_names
    ]
    concat_zeros = [np.zeros((B * s[0], *s[1:]), d) for s, d in zero_shapes]
    out_arrs = sharded(*concat_in, *concat_zeros)
    oi = out_names.index("out")
    res = np.asarray(out_arrs[oi]).reshape(B, *out_avals[oi].shape)
    return res.reshape(B, 1024)


# revision 32
# speedup vs baseline: 9.4047x; 3.2943x over previous
"""DGCNN forward on 8 Trainium2 NeuronCores, data-parallel over batch.

Per core (one point cloud, x [3, 2048]):
  4 edge-conv blocks, each:
    s[n,m] = 2*x_n.x_m - |x_m|^2            (augmented fp32 PE matmul; row-constant
                                             -|x_n|^2 dropped: rank-invariant per row)
    top-20 of each s row:  3 rounds of (max8, max_index, match_replace)
    x' = lrelu(max_k A[:, idx_k] + B)       (A = Wn@x, B = (Wc-Wn)@x; edge features
                                             never materialized: conv is linear and
                                             lrelu/max commute)
    The k=0 neighbor is always the point itself (s[n,n] is the row max), so its
    A-row comes from one contiguous DMA; only k=1..19 use indirect gathers.
    Block 4's A-table is bf16 (its output feeds only the global conv, no further
    knn), halving its gather traffic and enabling the 2x DVE mode for its k-max.
  then g = lrelu(Wg @ concat(x1..x4)), out = max_n g.

knn grams and blocks 1-3 stay exact fp32 (noise there corrupts the knn sets).
"""

import numpy as np
from contextlib import ExitStack

import concourse.bass as bass
import concourse.bacc as bacc
import concourse.mybir as mybir
from concourse.bass import IndirectOffsetOnAxis
from concourse.tile import TileContext

F32 = mybir.dt.float32
BF16 = mybir.dt.bfloat16
U16 = mybir.dt.uint16
U32 = mybir.dt.uint32

B, N, KNN, P = 8, 2048, 20, 128
NCHUNK = N // P            # 16
NEG = -3.0e38
SLOPE = 0.2
BLOCKS = [(3, 64), (64, 64), (64, 128), (128, 256)]
ACT = mybir.ActivationFunctionType
ALU = mybir.AluOpType


def build_nc():
    nc = bacc.Bacc("TRN2", target_bir_lowering=False)

    x_in = nc.dram_tensor("x", [3, N], F32, kind="ExternalInput")
    w_in = {}
    for bi, (c, o) in enumerate(BLOCKS):
        w_in[f"wnt{bi}"] = nc.dram_tensor(f"wnt{bi}", [c, o], F32, kind="ExternalInput")
        w_in[f"wdt{bi}"] = nc.dram_tensor(f"wdt{bi}", [c, o], F32, kind="ExternalInput")
    w_in["wgt"] = nc.dram_tensor("wgt", [512, 1024], F32, kind="ExternalInput")
    id_in = nc.dram_tensor("ident", [P, P], F32, kind="ExternalInput")
    out_d = nc.dram_tensor("out", [1024, 1], F32, kind="ExternalOutput")

    # DRAM scratch: per-block A^T feature tables
    at_dram = [
        nc.dram_tensor(f"at{bi}_scratch", [N, o], BF16 if bi == 3 else F32,
                       kind="Internal")
        for bi, (_, o) in enumerate(BLOCKS)
    ]

    with TileContext(nc) as tc, ExitStack() as ctx:
        ep = ctx.enter_context            # shorthand
        const = ep(tc.tile_pool(name="const", bufs=1))
        wpool = ep(tc.tile_pool(name="weights", bufs=1))
        xpool = ep(tc.tile_pool(name="xtiles", bufs=1))
        bpool = ep(tc.tile_pool(name="blockp", bufs=1))
        spool = ep(tc.tile_pool(name="sbuf", bufs=2))
        tkpool = ep(tc.tile_pool(name="topk", bufs=2))
        gpool = ep(tc.tile_pool(name="gather", bufs=2))
        perb = ep(tc.tile_pool(name="perblock", bufs=1))
        pp_s = ep(tc.tile_pool(name="ps_s", bufs=2, space="PSUM"))
        pp_m = ep(tc.tile_pool(name="ps_m", bufs=2, space="PSUM"))
        pp_t = ep(tc.tile_pool(name="ps_t", bufs=2, space="PSUM"))

        # block input x0 — first DMA: everything in block 1 hangs off it
        x0 = xpool.tile([3, N], F32)
        nc.sync.dma_start(out=x0[:], in_=x_in[:])

        # ---- constants ----
        ident = const.tile([P, P], F32)
        nc.sync.dma_start(out=ident[:], in_=id_in[:])
        ones_col = const.tile([P, 1], F32)
        nc.vector.memset(ones_col[:], 1.0)
        ones_row = const.tile([1, P], F32)
        nc.vector.memset(ones_row[:], 1.0)

        # ---- load pre-transposed weights (host supplies WnT/WdT/WgT) ----
        wnT, wdT = [], []
        for bi, (c, o) in enumerate(BLOCKS):
            wn = wpool.tile([c, o], F32, tag=f"wn{bi}")
            nc.sync.dma_start(out=wn[:], in_=w_in[f"wnt{bi}"][:])
            wd = wpool.tile([c, o], F32, tag=f"wd{bi}")
            nc.sync.dma_start(out=wd[:], in_=w_in[f"wdt{bi}"][:])
            wnT.append(wn)
            wdT.append(wd)

        # WgT loads are deferred into block 2 (first needed by block 4's gconv)
        wgT = [wpool.tile([P, 1024], F32, tag=f"wg{k}", name=f"wgT{k}") for k in range(4)]

        # ---- x_cat tiles (c12 assembled from x1t/x2t before the global conv) ----
        c12 = xpool.tile([P, N], F32)
        c3 = xpool.tile([P, N], F32)
        c4a = xpool.tile([P, N], F32)
        c4b = xpool.tile([P, N], F32)
        x1t = xpool.tile([64, N], F32)
        x2t = xpool.tile([64, N], F32)

        # global-conv partial maxima, filled as block 4 quarters complete
        redv_all = perb.tile([P, 32], F32, name="redv_all")

        def emit_gconv_unit(nq, oc):
            xcat = [c12, c3, c4a, c4b]
            pg = pp_s.tile([P, 512], F32, space="PSUM", tag="ph")
            for k in range(4):
                nc.tensor.matmul(out=pg[:], lhsT=wgT[k][:, oc * P:(oc + 1) * P],
                                 rhs=xcat[k][:, nq * 512:(nq + 1) * 512],
                                 start=(k == 0), stop=(k == 3))
            nc.vector.tensor_reduce(out=redv_all[:, oc * 4 + nq:oc * 4 + nq + 1],
                                    in_=pg[:], axis=mybir.AxisListType.X,
                                    op=ALU.max)

        def block_input(bi):
            return [x0[:], x1t[:], x2t[:], c3[:]][bi]

        def block_out_dst(bi):
            return [[x1t], [x2t], [c3], [c4a, c4b]][bi]

        gconv_pending = []

        # ================= edge blocks =================
        for bi, (C, O) in enumerate(BLOCKS):
            xb = block_input(bi)
            at_dt = BF16 if bi == 3 else F32

            if bi == 1:
                for k in range(4):
                    nc.sync.dma_start(out=wgT[k][:],
                                      in_=w_in["wgt"][k * P:(k + 1) * P, :])

            # --- per-block prep: xsq first (it feeds the negxx matmuls) ---
            xsq = bpool.tile([C, N], F32, tag="xsq")
            nc.scalar.activation(out=xsq[:], in_=xb, func=ACT.Square)
            x2 = bpool.tile([C, N], F32, tag="x2")
            nc.scalar.activation(out=x2[:], in_=xb, func=ACT.Copy, scale=2.0)
            negxx = bpool.tile([1, N], F32, tag="negxx")
            for q in range(4):
                mq = pp_m.tile([1, 512], F32, space="PSUM", tag="mm")
                nc.tensor.matmul(out=mq[:], lhsT=ones_col[:C, :], rhs=xsq[:, q * 512:(q + 1) * 512],
                                 start=True, stop=True)
                nc.scalar.activation(out=negxx[:, q * 512:(q + 1) * 512], in_=mq[:],
                                     func=ACT.Copy, scale=-1.0)

            def emit_at_table():
                # A^T rows to DRAM: at[n, :] = x_n . WnT  (chunk-wise).
                # Emitted after the first grams: it only gates the gathers.
                for i in range(NCHUNK):
                    ap_ = pp_m.tile([P, O], F32, space="PSUM", tag="mm")
                    nc.tensor.matmul(out=ap_[:], lhsT=xb[:, i * P:(i + 1) * P],
                                     rhs=wnT[bi][:], start=True, stop=True)
                    at_sb = spool.tile([P, O], at_dt, tag="at_sb")
                    nc.scalar.copy(out=at_sb[:], in_=ap_[:])
                    nc.sync.dma_start(out=at_dram[bi][i * P:(i + 1) * P, :], in_=at_sb[:])

            # augmented gram lhs/rhs for C <= 64 (single fused matmul); block4 separate
            if C <= 64:
                # augmented row must start at a 32-aligned partition; pad with zeros
                cpad = ((C + 31) // 32) * 32
                gl = bpool.tile([cpad + 1, N], F32, tag="gramL")
                gr = bpool.tile([cpad + 1, N], F32, tag="gramR")
                if cpad != C:
                    nc.vector.memset(gl[:], 0.0)
                    nc.vector.memset(gr[:], 0.0)
                nc.scalar.copy(out=gl[:C, :], in_=xb)
                nc.vector.memset(gl[cpad:cpad + 1, :], 1.0)
                nc.vector.tensor_copy(out=gr[:C, :], in_=x2[:])
                # engines are lane-aligned: partition 0 -> cpad needs a DMA
                nc.sync.dma_start(out=gr[cpad:cpad + 1, :], in_=negxx[:])

            # --- main chunk loop, split so chunk 0's top-k starts before the
            # at-table phase (which only the gathers depend on) ---
            def emit_front(i):
                # gram s[n, m] for n in chunk i : two psum halves [128, 1024]
                ps_h = []
                for h in range(2):
                    ph = pp_s.tile([P, 1024], F32, space="PSUM")
                    for q in range(2):
                        sl = slice((2 * h + q) * 512, (2 * h + q + 1) * 512)
                        if C <= 64:
                            nc.tensor.matmul(out=ph[:, q * 512:(q + 1) * 512],
                                             lhsT=gl[:, i * P:(i + 1) * P], rhs=gr[:, sl],
                                             start=True, stop=True)
                        else:
                            nc.tensor.matmul(out=ph[:, q * 512:(q + 1) * 512],
                                             lhsT=xb[:, i * P:(i + 1) * P], rhs=x2[:, sl],
                                             start=True, stop=False)
                            nc.tensor.matmul(out=ph[:, q * 512:(q + 1) * 512],
                                             lhsT=ones_row[:, :P],
                                             rhs=negxx[:, sl],
                                             start=False, stop=True)
                    ps_h.append(ph)

                # s -> SBUF (full row needed by the match-op scans)
                s_sb = tkpool.tile([P, N], F32, tag="s_sb")
                for h in range(2):
                    nc.scalar.copy(out=s_sb[:, h * 1024:(h + 1) * 1024], in_=ps_h[h][:])

                # exact top-20 of each row: 3 rounds of (max8, max_index,
                # in-place match_replace); s_sb is consumed
                # max_index writes u32 directly: each round's gathers stream
                # off its output with no staging copy (shortens the chunk tail
                # that gates each block boundary)
                v8 = tkpool.tile([P, 8], F32, tag="v8")
                nbr_a = tkpool.tile([P, 8], U32, tag="nbr_a")
                nbr_b = tkpool.tile([P, 8], U32, tag="nbr_b")
                nbr_c = tkpool.tile([P, 8], U32, tag="nbr_c")
                nc.vector.max(out=v8[:], in_=s_sb[:])
                nc.vector.max_index(out=nbr_a[:], in_max=v8[:], in_values=s_sb[:])
                nc.vector.match_replace(out=s_sb[:], in_to_replace=v8[:], in_values=s_sb[:],
                                        imm_value=NEG)
                nc.vector.max(out=v8[:], in_=s_sb[:])
                nc.vector.max_index(out=nbr_b[:], in_max=v8[:], in_values=s_sb[:])
                nc.vector.match_replace(out=s_sb[:], in_to_replace=v8[:], in_values=s_sb[:],
                                        imm_value=NEG)
                nc.vector.max(out=v8[:], in_=s_sb[:])
                nc.vector.max_index(out=nbr_c[:], in_max=v8[:], in_values=s_sb[:])
                return (nbr_a, nbr_b, nbr_c)

            def emit_back(i, idxs):
                nbr_a, nbr_b, nbr_c = idxs
                # gather neighbor A^T rows and reduce max over k.
                # k=0 is always the point itself (s[n,n] is the row max; for an
                # exact-duplicate point the A rows are identical), so it comes
                # from one contiguous DMA instead of an indirect gather.
                gath = gpool.tile([P, KNN, O], at_dt, tag="gath")
                nc.sync.dma_start(out=gath[:, 0, :],
                                  in_=at_dram[bi][i * P:(i + 1) * P, :])
                for j in range(1, KNN):
                    if j < 8:
                        off = nbr_a[:, j:j + 1]
                    elif j < 16:
                        off = nbr_b[:, j - 8:j - 7]
                    else:
                        off = nbr_c[:, j - 16:j - 15]
                    nc.gpsimd.indirect_dma_start(
                        out=gath[:, j, :], out_offset=None, in_=at_dram[bi][:],
                        in_offset=IndirectOffsetOnAxis(ap=off, axis=0))

                # k-max tree, in place: 20 -> 10 -> 5 -> (2,2,1) -> 1
                # (contiguous slices; the bf16 block gets the 2x DVE mode)
                g = gath[:]
                nc.vector.tensor_tensor(out=g[:, 0:10, :], in0=g[:, 0:10, :],
                                        in1=g[:, 10:20, :], op=ALU.max)
                nc.vector.tensor_tensor(out=g[:, 0:5, :], in0=g[:, 0:5, :],
                                        in1=g[:, 5:10, :], op=ALU.max)
                nc.vector.tensor_tensor(out=g[:, 0:2, :], in0=g[:, 0:2, :],
                                        in1=g[:, 2:4, :], op=ALU.max)
                nc.vector.tensor_tensor(out=g[:, 0:1, :], in0=g[:, 0:1, :],
                                        in1=g[:, 1:2, :], op=ALU.max)
                nc.vector.tensor_tensor(out=g[:, 0:1, :], in0=g[:, 0:1, :],
                                        in1=g[:, 4:5, :], op=ALU.max)

                # B^T chunk, add, leaky relu (fused max(v, 0.2v))
                bt = pp_m.tile([P, O], F32, space="PSUM", tag="mm")
                nc.tensor.matmul(out=bt[:], lhsT=xb[:, i * P:(i + 1) * P], rhs=wdT[bi][:],
                                 start=True, stop=True)
                xt = gpool.tile([P, O], F32, tag="xt")
                nc.vector.tensor_add(out=xt[:], in0=g[:, 0, :], in1=bt[:])
                nc.vector.scalar_tensor_tensor(out=xt[:], in0=xt[:], scalar=SLOPE,
                                               in1=xt[:], op0=ALU.mult, op1=ALU.max)

                # transpose back to [O, chunk] into the x_cat tiles
                dsts = block_out_dst(bi)
                for q in range((O + P - 1) // P):
                    osz = min(P, O - q * P)
                    tp = pp_t.tile([P, P], F32, space="PSUM")
                    nc.tensor.transpose(out=tp[:osz, :], in_=xt[:, q * P:q * P + osz],
                                        identity=ident[:])
                    nc.scalar.copy(out=dsts[q][0:osz, i * P:(i + 1) * P], in_=tp[:osz, :])

                # overlap the global conv with block 4: quarter i//4 of the
                # xcat columns is final after chunk 4*(i//4)+3; drain one
                # pending (nq, oc) unit into the PE slack of later chunks
                if bi == 3:
                    if i % 4 == 3:
                        gconv_pending.extend((i // 4, oc) for oc in range(8))
                    if i >= 4:
                        for _ in range(2):
                            if gconv_pending:
                                emit_gconv_unit(*gconv_pending.pop(0))

            i24_0 = emit_front(0)
            i24_1 = emit_front(1)
            emit_at_table()
            emit_back(0, i24_0)
            emit_back(1, i24_1)
            for i in range(2, NCHUNK):
                i24_i = emit_front(i)
                emit_back(i, i24_i)

            if bi == 1:
                # assemble c12 = [x1; x2] for the global conv
                nc.sync.dma_start(out=c12[0:64, :], in_=x1t[:])
                nc.sync.dma_start(out=c12[64:128, :], in_=x2t[:])

        # ================= global conv + max (mostly interleaved above) ======
        while gconv_pending:
            emit_gconv_unit(*gconv_pending.pop(0))
        for oc in range(8):
            red1 = spool.tile([P, 1], F32, tag="red1")
            nc.vector.tensor_reduce(out=red1[:], in_=redv_all[:, oc * 4:(oc + 1) * 4],
                                    axis=mybir.AxisListType.X, op=ALU.max)
            nc.vector.scalar_tensor_tensor(out=red1[:], in0=red1[:], scalar=SLOPE,
                                           in1=red1[:], op0=ALU.mult, op1=ALU.max)
            nc.sync.dma_start(out=out_d[oc * P:(oc + 1) * P, :], in_=red1[:])

    nc.compile()
    return nc


_NC_CACHE = None


def _get_nc():
    global _NC_CACHE
    if _NC_CACHE is None:
        _NC_CACHE = build_nc()
    return _NC_CACHE


def host_weights(W1, W2, W3, W4, Wg):
    ws = {}
    for bi, (wm, (c, o)) in enumerate(zip([W1, W2, W3, W4], BLOCKS)):
        wm = np.asarray(wm, dtype=np.float32)
        wn = wm[:, :c]
        wd = wm[:, c:] - wn
        ws[f"wnt{bi}"] = np.ascontiguousarray(wn.T)
        ws[f"wdt{bi}"] = np.ascontiguousarray(wd.T)
    ws["wgt"] = np.ascontiguousarray(np.asarray(Wg, dtype=np.float32).T)
    ws["ident"] = np.eye(P, dtype=np.float32)
    return ws


_EXEC_CACHE = None


def _get_exec(nc):
    """Build the 8-core shard_map executable once (mirrors
    bass2jax.run_bass_via_pjrt) so repeat kernel() calls skip retracing."""
    global _EXEC_CACHE
    if _EXEC_CACHE is not None:
        return _EXEC_CACHE
    import jax
    from jax.sharding import Mesh, PartitionSpec
    from jax.experimental.shard_map import shard_map
    from concourse import bass2jax

    bass2jax.install_neuronx_cc_hook()
    assert nc.dbg_addr is None
    partition_name = nc.partition_id_tensor.name if nc.partition_id_tensor else None
    in_names, out_names, out_avals, zero_shapes = [], [], [], []
    for alloc in nc.m.functions[0].allocations:
        if not isinstance(alloc, mybir.MemoryLocationSet):
            continue
        name = alloc.memorylocations[0].name
        if alloc.kind == "ExternalInput":
            if name != partition_name:
                in_names.append(name)
        elif alloc.kind == "ExternalOutput":
            out_names.append(name)
            shape = tuple(alloc.tensor_shape)
            dtype = mybir.dt.np(alloc.dtype)
            out_avals.append(jax.core.ShapedArray(shape, dtype))
            zero_shapes.append((shape, dtype))
    n_params = len(in_names)
    n_outs = len(out_avals)
    all_names = in_names + out_names
    if partition_name is not None:
        all_names.append(partition_name)
    donate = tuple(range(n_params, n_params + n_outs))

    def _body(*args):
        operands = list(args)
        if partition_name is not None:
            operands.append(bass2jax.partition_id_tensor())
        outs = bass2jax._bass_exec_p.bind(
            *operands,
            out_avals=tuple(out_avals),
            in_names=tuple(all_names),
            out_names=tuple(out_names),
            lowering_input_output_aliases=(),
            sim_require_finite=True,
            sim_require_nnan=True,
            nc=nc,
        )
        return tuple(outs)

    devices = jax.devices()[:B]
    mesh = Mesh(np.asarray(devices), ("core",))
    sharding = jax.sharding.NamedSharding(mesh, PartitionSpec("core"))
    sharded = jax.jit(
        shard_map(_body, mesh=mesh,
                  in_specs=(PartitionSpec("core"),) * (n_params + n_outs),
                  out_specs=(PartitionSpec("core"),) * n_outs,
                  check_rep=False),
        donate_argnums=donate, keep_unused=True,
    )
    _EXEC_CACHE = (sharded, in_names, out_names, out_avals, zero_shapes, sharding)
    return _EXEC_CACHE


_WCACHE = {}


def _dev_cached(name, arr, sharding):
    """Device-resident cache for per-call-identical inputs (weights etc.);
    inputs are not donated, so the committed arrays are reusable."""
    import hashlib
    import jax
    h = hashlib.md5(arr.tobytes()).hexdigest()
    ent = _WCACHE.get(name)
    if ent is not None and ent[0] == h:
        return ent[1]
    dev = jax.device_put(arr, sharding)
    _WCACHE[name] = (h, dev)
    return dev


def kernel(x, W1, W2, W3, W4, Wg):
    nc = _get_nc()
    sharded, in_names, out_names, out_avals, zero_shapes, sharding = _get_exec(nc)
    x = np.asarray(x, dtype=np.float32)
    ws = host_weights(W1, W2, W3, W4, Wg)
    concat_in = []
    for name in in_names:
        if name == "x":
            concat_in.append(np.concatenate(
                [np.ascontiguousarray(x[b]) for b in range(B)], axis=0))
        else:
            rep = np.concatenate([np.asarray(ws[name])] * B, axis=0)
            concat_in.append(_dev_cached(name, rep, sharding))
    concat_zeros = [np.zeros((B * s[0], *s[1:]), d) for s, d in zero_shapes]
    out_arrs = sharded(*concat_in, *concat_zeros)
    oi = out_names.index("out")
    res = np.asarray(out_arrs[oi]).reshape(B, *out_avals[oi].shape)
    return res.reshape(B, 1024)


# revision 33
# speedup vs baseline: 20.3420x; 2.1630x over previous
"""DGCNN forward on 8 Trainium2 NeuronCores, data-parallel over batch.

Per core (one point cloud, x [3, 2048]):
  4 edge-conv blocks, each:
    s[n,m] = 2*x_n.x_m - |x_m|^2            (augmented fp32 PE matmul; row-constant
                                             -|x_n|^2 dropped: rank-invariant per row)
    top-20 of each s row:  3 rounds of (max8, max_index, match_replace)
    x' = lrelu(max_k A[:, idx_k] + B)       (A = Wn@x, B = (Wc-Wn)@x; edge features
                                             never materialized: conv is linear and
                                             lrelu/max commute)
    The k=0 neighbor is always the point itself (s[n,n] is the row max), so its
    A-row comes from one contiguous DMA; only k=1..19 use indirect gathers.
    Block 4's A-table is bf16 (its output feeds only the global conv, no further
    knn), halving its gather traffic and enabling the 2x DVE mode for its k-max.
  then g = lrelu(Wg @ concat(x1..x4)), out = max_n g.

knn grams and blocks 1-3 stay exact fp32 (noise there corrupts the knn sets).
"""

import numpy as np
from contextlib import ExitStack

import concourse.bass as bass
import concourse.bacc as bacc
import concourse.mybir as mybir
from concourse.bass import IndirectOffsetOnAxis
from concourse.tile import TileContext

F32 = mybir.dt.float32
BF16 = mybir.dt.bfloat16
U16 = mybir.dt.uint16
U32 = mybir.dt.uint32

B, N, KNN, P = 8, 2048, 20, 128
NCHUNK = N // P            # 16
NEG = -3.0e38
SLOPE = 0.2
BLOCKS = [(3, 64), (64, 64), (64, 128), (128, 256)]
ACT = mybir.ActivationFunctionType
ALU = mybir.AluOpType


def build_nc():
    nc = bacc.Bacc("TRN2", target_bir_lowering=False)

    x_in = nc.dram_tensor("x", [3, N], F32, kind="ExternalInput")
    w_in = {}
    for bi, (c, o) in enumerate(BLOCKS):
        w_in[f"wnt{bi}"] = nc.dram_tensor(f"wnt{bi}", [c, o], F32, kind="ExternalInput")
        w_in[f"wdt{bi}"] = nc.dram_tensor(f"wdt{bi}", [c, o], F32, kind="ExternalInput")
    w_in["wgt"] = nc.dram_tensor("wgt", [512, 1024], F32, kind="ExternalInput")
    id_in = nc.dram_tensor("ident", [P, P], F32, kind="ExternalInput")
    out_d = nc.dram_tensor("out", [1024, 1], F32, kind="ExternalOutput")

    # DRAM scratch: per-block A^T feature tables
    at_dram = [
        nc.dram_tensor(f"at{bi}_scratch", [N, o], BF16 if bi == 3 else F32,
                       kind="Internal")
        for bi, (_, o) in enumerate(BLOCKS)
    ]

    with TileContext(nc) as tc, ExitStack() as ctx:
        ep = ctx.enter_context            # shorthand
        const = ep(tc.tile_pool(name="const", bufs=1))
        wpool = ep(tc.tile_pool(name="weights", bufs=1))
        xpool = ep(tc.tile_pool(name="xtiles", bufs=1))
        bpool = ep(tc.tile_pool(name="blockp", bufs=1))
        spool = ep(tc.tile_pool(name="sbuf", bufs=2))
        tkpool = ep(tc.tile_pool(name="topk", bufs=2))
        gpool = ep(tc.tile_pool(name="gather", bufs=2))
        perb = ep(tc.tile_pool(name="perblock", bufs=1))
        pp_s = ep(tc.tile_pool(name="ps_s", bufs=2, space="PSUM"))
        pp_m = ep(tc.tile_pool(name="ps_m", bufs=2, space="PSUM"))
        pp_t = ep(tc.tile_pool(name="ps_t", bufs=2, space="PSUM"))

        # block input x0 — first DMA: everything in block 1 hangs off it
        x0 = xpool.tile([3, N], F32)
        nc.sync.dma_start(out=x0[:], in_=x_in[:])

        # ---- constants ----
        ident = const.tile([P, P], F32)
        nc.sync.dma_start(out=ident[:], in_=id_in[:])
        ones_col = const.tile([P, 1], F32)
        nc.vector.memset(ones_col[:], 1.0)
        ones_row = const.tile([1, P], F32)
        nc.vector.memset(ones_row[:], 1.0)

        # ---- load pre-transposed weights (host supplies WnT/WdT/WgT) ----
        wnT, wdT = [], []
        for bi, (c, o) in enumerate(BLOCKS):
            wn = wpool.tile([c, o], F32, tag=f"wn{bi}")
            nc.sync.dma_start(out=wn[:], in_=w_in[f"wnt{bi}"][:])
            wd = wpool.tile([c, o], F32, tag=f"wd{bi}")
            nc.sync.dma_start(out=wd[:], in_=w_in[f"wdt{bi}"][:])
            wnT.append(wn)
            wdT.append(wd)

        # WgT loads are deferred into block 2 (first needed by block 4's gconv)
        wgT = [wpool.tile([P, 1024], F32, tag=f"wg{k}", name=f"wgT{k}") for k in range(4)]

        # ---- x_cat tiles (c12 assembled from x1t/x2t before the global conv) ----
        c12 = xpool.tile([P, N], F32)
        c3 = xpool.tile([P, N], F32)
        c4a = xpool.tile([P, N], F32)
        c4b = xpool.tile([P, N], F32)
        x1t = xpool.tile([64, N], F32)
        x2t = xpool.tile([64, N], F32)

        # global-conv partial maxima, filled as block 4 quarters complete
        redv_all = perb.tile([P, 32], F32, name="redv_all")

        def emit_gconv_unit(nq, oc):
            xcat = [c12, c3, c4a, c4b]
            pg = pp_s.tile([P, 512], F32, space="PSUM", tag="ph")
            for k in range(4):
                nc.tensor.matmul(out=pg[:], lhsT=wgT[k][:, oc * P:(oc + 1) * P],
                                 rhs=xcat[k][:, nq * 512:(nq + 1) * 512],
                                 start=(k == 0), stop=(k == 3))
            nc.vector.tensor_reduce(out=redv_all[:, oc * 4 + nq:oc * 4 + nq + 1],
                                    in_=pg[:], axis=mybir.AxisListType.X,
                                    op=ALU.max)

        def block_input(bi):
            return [x0[:], x1t[:], x2t[:], c3[:]][bi]

        def block_out_dst(bi):
            return [[x1t], [x2t], [c3], [c4a, c4b]][bi]

        gconv_pending = []

        # ================= edge blocks =================
        for bi, (C, O) in enumerate(BLOCKS):
            xb = block_input(bi)
            at_dt = BF16 if bi == 3 else F32

            if bi == 1:
                for k in range(4):
                    nc.sync.dma_start(out=wgT[k][:],
                                      in_=w_in["wgt"][k * P:(k + 1) * P, :])

            # --- per-block prep: xsq first (it feeds the negxx matmuls) ---
            xsq = bpool.tile([C, N], F32, tag="xsq")
            nc.scalar.activation(out=xsq[:], in_=xb, func=ACT.Square)
            x2 = bpool.tile([C, N], F32, tag="x2")
            nc.scalar.activation(out=x2[:], in_=xb, func=ACT.Copy, scale=2.0)
            negxx = bpool.tile([1, N], F32, tag="negxx")
            for q in range(4):
                mq = pp_m.tile([1, 512], F32, space="PSUM", tag="mm")
                nc.tensor.matmul(out=mq[:], lhsT=ones_col[:C, :], rhs=xsq[:, q * 512:(q + 1) * 512],
                                 start=True, stop=True)
                nc.scalar.activation(out=negxx[:, q * 512:(q + 1) * 512], in_=mq[:],
                                     func=ACT.Copy, scale=-1.0)

            def emit_at_table():
                # A^T rows to DRAM: at[n, :] = x_n . WnT  (chunk-wise).
                # Emitted after the first grams: it only gates the gathers.
                for i in range(NCHUNK):
                    ap_ = pp_m.tile([P, O], F32, space="PSUM", tag="mm")
                    nc.tensor.matmul(out=ap_[:], lhsT=xb[:, i * P:(i + 1) * P],
                                     rhs=wnT[bi][:], start=True, stop=True)
                    at_sb = spool.tile([P, O], at_dt, tag="at_sb")
                    nc.scalar.copy(out=at_sb[:], in_=ap_[:])
                    nc.sync.dma_start(out=at_dram[bi][i * P:(i + 1) * P, :], in_=at_sb[:])

            # augmented gram lhs/rhs for C <= 64 (single fused matmul); block4 separate
            if C <= 64:
                # augmented row must start at a 32-aligned partition; pad with zeros
                cpad = ((C + 31) // 32) * 32
                gl = bpool.tile([cpad + 1, N], F32, tag="gramL")
                gr = bpool.tile([cpad + 1, N], F32, tag="gramR")
                if cpad != C:
                    nc.vector.memset(gl[:], 0.0)
                    nc.vector.memset(gr[:], 0.0)
                nc.scalar.copy(out=gl[:C, :], in_=xb)
                nc.vector.memset(gl[cpad:cpad + 1, :], 1.0)
                nc.vector.tensor_copy(out=gr[:C, :], in_=x2[:])
                # engines are lane-aligned: partition 0 -> cpad needs a DMA
                nc.sync.dma_start(out=gr[cpad:cpad + 1, :], in_=negxx[:])

            # --- main chunk loop, split so chunk 0's top-k starts before the
            # at-table phase (which only the gathers depend on) ---
            def emit_front(i):
                # gram s[n, m] for n in chunk i : two psum halves [128, 1024]
                ps_h = []
                for h in range(2):
                    ph = pp_s.tile([P, 1024], F32, space="PSUM")
                    for q in range(2):
                        sl = slice((2 * h + q) * 512, (2 * h + q + 1) * 512)
                        if C <= 64:
                            nc.tensor.matmul(out=ph[:, q * 512:(q + 1) * 512],
                                             lhsT=gl[:, i * P:(i + 1) * P], rhs=gr[:, sl],
                                             start=True, stop=True)
                        else:
                            nc.tensor.matmul(out=ph[:, q * 512:(q + 1) * 512],
                                             lhsT=xb[:, i * P:(i + 1) * P], rhs=x2[:, sl],
                                             start=True, stop=False)
                            nc.tensor.matmul(out=ph[:, q * 512:(q + 1) * 512],
                                             lhsT=ones_row[:, :P],
                                             rhs=negxx[:, sl],
                                             start=False, stop=True)
                    ps_h.append(ph)

                # s -> SBUF (full row needed by the match-op scans)
                s_sb = tkpool.tile([P, N], F32, tag="s_sb")
                for h in range(2):
                    nc.scalar.copy(out=s_sb[:, h * 1024:(h + 1) * 1024], in_=ps_h[h][:])

                # exact top-20 of each row: 3 rounds of (max8, max_index,
                # in-place match_replace); s_sb is consumed
                # max_index writes u32 directly: each round's gathers stream
                # off its output with no staging copy (shortens the chunk tail
                # that gates each block boundary)
                v8 = tkpool.tile([P, 8], F32, tag="v8")
                nbr_a = tkpool.tile([P, 8], U32, tag="nbr_a")
                nbr_b = tkpool.tile([P, 8], U32, tag="nbr_b")
                nbr_c = tkpool.tile([P, 8], U32, tag="nbr_c")
                nc.vector.max(out=v8[:], in_=s_sb[:])
                nc.vector.max_index(out=nbr_a[:], in_max=v8[:], in_values=s_sb[:])
                nc.vector.match_replace(out=s_sb[:], in_to_replace=v8[:], in_values=s_sb[:],
                                        imm_value=NEG)
                nc.vector.max(out=v8[:], in_=s_sb[:])
                nc.vector.max_index(out=nbr_b[:], in_max=v8[:], in_values=s_sb[:])
                nc.vector.match_replace(out=s_sb[:], in_to_replace=v8[:], in_values=s_sb[:],
                                        imm_value=NEG)
                nc.vector.max(out=v8[:], in_=s_sb[:])
                nc.vector.max_index(out=nbr_c[:], in_max=v8[:], in_values=s_sb[:])
                return (nbr_a, nbr_b, nbr_c)

            def emit_back(i, idxs):
                nbr_a, nbr_b, nbr_c = idxs
                # gather neighbor A^T rows and reduce max over k.
                # k=0 is always the point itself (s[n,n] is the row max; for an
                # exact-duplicate point the A rows are identical), so it comes
                # from one contiguous DMA instead of an indirect gather.
                gath = gpool.tile([P, KNN, O], at_dt, tag="gath")
                nc.sync.dma_start(out=gath[:, 0, :],
                                  in_=at_dram[bi][i * P:(i + 1) * P, :])
                for j in range(1, KNN):
                    if j < 8:
                        off = nbr_a[:, j:j + 1]
                    elif j < 16:
                        off = nbr_b[:, j - 8:j - 7]
                    else:
                        off = nbr_c[:, j - 16:j - 15]
                    nc.gpsimd.indirect_dma_start(
                        out=gath[:, j, :], out_offset=None, in_=at_dram[bi][:],
                        in_offset=IndirectOffsetOnAxis(ap=off, axis=0))

                # k-max tree, in place: 20 -> 10 -> 5 -> (2,2,1) -> 1
                # (contiguous slices; the bf16 block gets the 2x DVE mode)
                g = gath[:]
                nc.vector.tensor_tensor(out=g[:, 0:10, :], in0=g[:, 0:10, :],
                                        in1=g[:, 10:20, :], op=ALU.max)
                nc.vector.tensor_tensor(out=g[:, 0:5, :], in0=g[:, 0:5, :],
                                        in1=g[:, 5:10, :], op=ALU.max)
                nc.vector.tensor_tensor(out=g[:, 0:2, :], in0=g[:, 0:2, :],
                                        in1=g[:, 2:4, :], op=ALU.max)
                nc.vector.tensor_tensor(out=g[:, 0:1, :], in0=g[:, 0:1, :],
                                        in1=g[:, 1:2, :], op=ALU.max)
                nc.vector.tensor_tensor(out=g[:, 0:1, :], in0=g[:, 0:1, :],
                                        in1=g[:, 4:5, :], op=ALU.max)

                # B^T chunk, add, leaky relu (fused max(v, 0.2v))
                bt = pp_m.tile([P, O], F32, space="PSUM", tag="mm")
                nc.tensor.matmul(out=bt[:], lhsT=xb[:, i * P:(i + 1) * P], rhs=wdT[bi][:],
                                 start=True, stop=True)
                xt = gpool.tile([P, O], F32, tag="xt")
                nc.vector.tensor_add(out=xt[:], in0=g[:, 0, :], in1=bt[:])
                nc.vector.scalar_tensor_tensor(out=xt[:], in0=xt[:], scalar=SLOPE,
                                               in1=xt[:], op0=ALU.mult, op1=ALU.max)

                # transpose back to [O, chunk] into the x_cat tiles
                dsts = block_out_dst(bi)
                for q in range((O + P - 1) // P):
                    osz = min(P, O - q * P)
                    tp = pp_t.tile([P, P], F32, space="PSUM")
                    nc.tensor.transpose(out=tp[:osz, :], in_=xt[:, q * P:q * P + osz],
                                        identity=ident[:])
                    nc.scalar.copy(out=dsts[q][0:osz, i * P:(i + 1) * P], in_=tp[:osz, :])

                # overlap the global conv with block 4: quarter i//4 of the
                # xcat columns is final after chunk 4*(i//4)+3; drain one
                # pending (nq, oc) unit into the PE slack of later chunks
                if bi == 3:
                    if i % 4 == 3:
                        gconv_pending.extend((i // 4, oc) for oc in range(8))
                    if i >= 4:
                        for _ in range(2):
                            if gconv_pending:
                                emit_gconv_unit(*gconv_pending.pop(0))

            i24_0 = emit_front(0)
            i24_1 = emit_front(1)
            emit_at_table()
            emit_back(0, i24_0)
            emit_back(1, i24_1)
            for i in range(2, NCHUNK):
                i24_i = emit_front(i)
                emit_back(i, i24_i)

            if bi == 1:
                # assemble c12 = [x1; x2] for the global conv
                nc.sync.dma_start(out=c12[0:64, :], in_=x1t[:])
                nc.sync.dma_start(out=c12[64:128, :], in_=x2t[:])

        # ================= global conv + max (mostly interleaved above) ======
        while gconv_pending:
            emit_gconv_unit(*gconv_pending.pop(0))
        for oc in range(8):
            red1 = spool.tile([P, 1], F32, tag="red1")
            nc.vector.tensor_reduce(out=red1[:], in_=redv_all[:, oc * 4:(oc + 1) * 4],
                                    axis=mybir.AxisListType.X, op=ALU.max)
            nc.vector.scalar_tensor_tensor(out=red1[:], in0=red1[:], scalar=SLOPE,
                                           in1=red1[:], op0=ALU.mult, op1=ALU.max)
            nc.sync.dma_start(out=out_d[oc * P:(oc + 1) * P, :], in_=red1[:])

    nc.compile()
    return nc


_NC_CACHE = None


def _get_nc():
    global _NC_CACHE
    if _NC_CACHE is None:
        _NC_CACHE = build_nc()
    return _NC_CACHE


def host_weights(W1, W2, W3, W4, Wg):
    ws = {}
    for bi, (wm, (c, o)) in enumerate(zip([W1, W2, W3, W4], BLOCKS)):
        wm = np.asarray(wm, dtype=np.float32)
        wn = wm[:, :c]
        wd = wm[:, c:] - wn
        ws[f"wnt{bi}"] = np.ascontiguousarray(wn.T)
        ws[f"wdt{bi}"] = np.ascontiguousarray(wd.T)
    ws["wgt"] = np.ascontiguousarray(np.asarray(Wg, dtype=np.float32).T)
    ws["ident"] = np.eye(P, dtype=np.float32)
    return ws


_EXEC_CACHE = None


def _get_exec(nc):
    """Build the 8-core shard_map executable once (mirrors
    bass2jax.run_bass_via_pjrt) so repeat kernel() calls skip retracing."""
    global _EXEC_CACHE
    if _EXEC_CACHE is not None:
        return _EXEC_CACHE
    import jax
    from jax.sharding import Mesh, PartitionSpec
    from jax.experimental.shard_map import shard_map
    from concourse import bass2jax

    bass2jax.install_neuronx_cc_hook()
    assert nc.dbg_addr is None
    partition_name = nc.partition_id_tensor.name if nc.partition_id_tensor else None
    in_names, out_names, out_avals, zero_shapes = [], [], [], []
    for alloc in nc.m.functions[0].allocations:
        if not isinstance(alloc, mybir.MemoryLocationSet):
            continue
        name = alloc.memorylocations[0].name
        if alloc.kind == "ExternalInput":
            if name != partition_name:
                in_names.append(name)
        elif alloc.kind == "ExternalOutput":
            out_names.append(name)
            shape = tuple(alloc.tensor_shape)
            dtype = mybir.dt.np(alloc.dtype)
            out_avals.append(jax.core.ShapedArray(shape, dtype))
            zero_shapes.append((shape, dtype))
    n_params = len(in_names)
    n_outs = len(out_avals)
    all_names = in_names + out_names
    if partition_name is not None:
        all_names.append(partition_name)
    donate = tuple(range(n_params, n_params + n_outs))

    def _body(*args):
        operands = list(args)
        if partition_name is not None:
            operands.append(bass2jax.partition_id_tensor())
        outs = bass2jax._bass_exec_p.bind(
            *operands,
            out_avals=tuple(out_avals),
            in_names=tuple(all_names),
            out_names=tuple(out_names),
            lowering_input_output_aliases=(),
            sim_require_finite=True,
            sim_require_nnan=True,
            nc=nc,
        )
        return tuple(outs)

    devices = jax.devices()[:B]
    mesh = Mesh(np.asarray(devices), ("core",))
    sharding = jax.sharding.NamedSharding(mesh, PartitionSpec("core"))
    sharded = jax.jit(
        shard_map(_body, mesh=mesh,
                  in_specs=(PartitionSpec("core"),) * (n_params + n_outs),
                  out_specs=(PartitionSpec("core"),) * n_outs,
                  check_rep=False),
        donate_argnums=donate, keep_unused=True,
    )
    _EXEC_CACHE = (sharded, in_names, out_names, out_avals, zero_shapes, sharding)
    return _EXEC_CACHE


_WCACHE = {}


def kernel(x, W1, W2, W3, W4, Wg):
    nc = _get_nc()
    sharded, in_names, out_names, out_avals, zero_shapes, sharding = _get_exec(nc)
    import hashlib
    import jax

    x = np.ascontiguousarray(np.asarray(x, dtype=np.float32))
    # digest the raw (unreplicated) weights; rebuild + re-upload only on change
    h = hashlib.md5()
    for w in (W1, W2, W3, W4, Wg):
        h.update(np.ascontiguousarray(np.asarray(w, dtype=np.float32)).tobytes())
    key = h.hexdigest()
    ent = _WCACHE.get("w")
    if ent is None or ent[0] != key:
        ws = host_weights(W1, W2, W3, W4, Wg)
        devs = {
            name: jax.device_put(
                np.concatenate([np.asarray(ws[name])] * B, axis=0), sharding)
            for name in in_names if name != "x"
        }
        zeros = [np.zeros((B * s[0], *s[1:]), d) for s, d in zero_shapes]
        _WCACHE["w"] = (key, devs, zeros)
    _, devs, concat_zeros = _WCACHE["w"]
    concat_in = [x.reshape(B * 3, N) if name == "x" else devs[name]
                 for name in in_names]
    out_arrs = sharded(*concat_in, *concat_zeros)
    oi = out_names.index("out")
    res = np.asarray(out_arrs[oi]).reshape(B, *out_avals[oi].shape)
    return res.reshape(B, 1024)
